# revision 22
# baseline (speedup 1.0000x reference)
"""Multi-head attention forward (B=4, L=2048, E=1024, H=16) on 8 NeuronCores.

Sharding: core c handles batch b = c // 2 and head-group g = c % 2 (8 heads,
512 embed dims). Each core computes its QKV projections, attention, and a
partial out-projection over its 512 contraction dims; the host sums the two
partials per batch and adds the bias.

All transposes and bf16 casts happen on the host: each core receives
xqT/xkT/xvT as [E, L] bf16, wqkvT as [E, 3*FG] bf16 (cols q|k|v) and
woutT as [FG, E] bf16.

Schedule: the ACT engine's exp stream (one [128, 512*GRP] activation per
score group) is the critical resource (~270us); every projection
(k/q/v/out) is broken into 8/16-matmul bursts injected between attention
groups so the tensor engine's spare capacity under the exp cadence is
used and ACT never idles at iteration boundaries. All input staging uses
single multi-dim DMA descriptors (the sync queue issues ~0.6us per
descriptor, so per-chunk DMAs would gate the whole prolog).

Self-contained: only needs numpy + the concourse stack at /opt/trn_rl_repo.
"""

import sys

import numpy as np

sys.path.insert(0, "/opt/trn_rl_repo")

import ml_dtypes  # noqa: E402

import concourse.bass as bass  # noqa: E402
import concourse.tile as tile  # noqa: E402
from concourse import bacc, mybir  # noqa: E402
from concourse import bass_utils  # noqa: E402

F32 = mybir.dt.float32
BF16 = mybir.dt.bfloat16
EXP = mybir.ActivationFunctionType.Exp
NP_BF16 = ml_dtypes.bfloat16

P = 128          # partitions
L = 2048         # sequence length
E = 1024         # embed dim
FG = 512         # per-core feature slice (8 heads x 64)
D = 64           # head dim
EC = E // P      # 8 e-chunks (contraction tiles for projections)
SC = L // P      # 16 s-chunks
LG = L // 512    # 4 q-windows of 512
FT = FG // P     # 4 head pairs
GRP = 3          # score psum banks per exp group
NU = 2 * SC      # 32 (sc, head) units per (p, lg)
W3 = 3 * FG      # wqkv row width (1536)


def _build():
    nc = bacc.Bacc("TRN2", target_bir_lowering=False, debug=False, num_devices=8)

    # host-prearranged layouts: x* are [P, LG*EC*512] window-major
    # (cols = w*4096 + ec*512 + c), wq is slice-major (see _shard_inputs),
    # wo is [P, FT*E] ec-major. All DMAs are fully contiguous per partition.
    xqT_d = nc.dram_tensor("xqh", [P, EC * L], BF16, kind="ExternalInput")
    xkT_d = nc.dram_tensor("xkh", [P, EC * L], BF16, kind="ExternalInput")
    xvT_d = nc.dram_tensor("xvh", [P, EC * L], BF16, kind="ExternalInput")
    wqkvT_d = nc.dram_tensor("wqh", [P, EC * W3], BF16, kind="ExternalInput")
    woutT_d = nc.dram_tensor("woh", [P, FT * E], BF16, kind="ExternalInput")
    out_d = nc.dram_tensor("out", [L, E], BF16, kind="ExternalOutput")

    with tile.TileContext(nc) as tc:
        with (
            tc.tile_pool(name="const", bufs=1) as constp,
            tc.tile_pool(name="pers", bufs=1) as pers,
            tc.tile_pool(name="xin", bufs=1) as xin,
            tc.tile_pool(name="xv", bufs=2) as xvp,
            tc.tile_pool(name="xq", bufs=2) as xqp,
            tc.tile_pool(name="stage", bufs=2) as stage,
            tc.tile_pool(name="ps", bufs=2, space="PSUM") as psp,
            tc.tile_pool(name="psav", bufs=1, space="PSUM") as psav,
        ):
            # engine warm-ups (prime DVE cast path + preload the EXP table)
            warm32 = constp.tile([P, 16], F32, tag="warm32", name="warm32")
            nc.vector.memset(warm32[:], 0.0)
            warm16 = constp.tile([P, 16], BF16, tag="warm16", name="warm16")
            nc.vector.tensor_copy(warm16[:], warm32[:])
            warmE = constp.tile([P, 16], BF16, tag="warmE", name="warmE")
            nc.scalar.activation(warmE[:], warm32[:], EXP, scale=0.125)
            warmG = constp.tile([P, 16], F32, tag="warmG", name="warmG")
            nc.gpsimd.memset(warmG[:], 0.0)
            # reciprocal staging: rows 0/32 hold denominators per tail, the
            # rest stays 1.0 so the batched [64,512] reciprocal is stable
            rr = constp.tile([P, 512], F32, tag="rr", name="rr")
            nc.vector.memset(rr[:], 1.0)

            # persistent activations / weights (ec-major fused layouts)
            kT = [pers.tile([P, L], BF16, tag=f"kT{p}", name=f"kT{p}")
                  for p in range(FT)]
            qT = [pers.tile([P, L], BF16, tag=f"qT{p}", name=f"qT{p}")
                  for p in range(FT)]
            avN = [pers.tile([P, L], BF16, tag=f"avN{p}", name=f"avN{p}")
                   for p in range(FT)]
            # AV stationary tiles: per s-chunk, 4 pairs x 256 cols:
            #   [v_h0(64) | ones(1) | junk(63)]  -> av rows 0:64, sum row 64
            #   [junk(32) | ones(1) | junk(31) | v_h1(64)] -> rows 64:128, sum row 32
            vst = [pers.tile([P, 1024], BF16, tag=f"vst{s}", name=f"vst{s}")
                   for s in range(SC)]
            wqA = pers.tile([P, EC * W3], BF16, tag="wqA", name="wqA")
            woA = pers.tile([P, FT * E], BF16, tag="woA", name="woA")
            xkA = xin.tile([P, EC * L], BF16, tag="xkA", name="xkA")

            # ---- fused DMA helpers (one descriptor per logical transfer);
            # eng selects the issuing engine = the hardware DMA queue, so
            # independent streams transfer in parallel ----
            # wq slice-major blocks (host lays out identically):
            #   [0:1024]      wk p=0   (ec-major, 128/ec)
            #   [1024:2048]   wq p=0   (ec-major, 128/ec)
            #   [2048:5120]   wk p=1-3 (ec-major, 384/ec)
            #   [5120:8192]   wq p=1-3 (ec-major, 384/ec)
            #   [8192:12288]  wv       (ec-major, 512/ec)
            W_SL = {(1, 0): (0, 1024), (0, 0): (1024, 1024),
                    (1, 1): (2048, 3072), (0, 1): (5120, 3072),
                    (2, 0): (8192, 4096)}

            def wk_col(ec, p):
                if p == 0:
                    return 0 + ec * P
                return 2048 + ec * 384 + (p - 1) * P

            def wq_col(ec, p):
                if p == 0:
                    return 1024 + ec * P
                return 5120 + ec * 384 + (p - 1) * P

            def wv_col(ec):
                return 8192 + ec * FG

            def dma_w_slice(qkv, p0, p1, eng):
                c0, n = W_SL[(qkv, p0)]
                s = wqkvT_d.ap()
                eng.dma_start(wqA[:, c0:c0 + n], s[0:P, c0:c0 + n])

            def dma_xk_window(w):
                s = xkT_d.ap()
                nc.sync.dma_start(
                    xkA[:, w * 4096:(w + 1) * 4096],
                    s[0:P, w * 4096:(w + 1) * 4096])

            def dma_wout():
                s = woutT_d.ap()
                nc.scalar.dma_start(woA[:], s[0:P, :])

            xqwin = {}

            def dma_xq_window(lg):
                t = xqp.tile([P, EC * 512], BF16, tag="xqw", name="xqw")
                s = xqT_d.ap()
                nc.gpsimd.dma_start(t[:], s[0:P, lg * 4096:(lg + 1) * 4096])
                xqwin[lg] = t

            xvw = {}

            def dma_xv_window(w, eng):
                t = xvp.tile([P, EC * 512], BF16, tag="xvw", name="xvw")
                s = xvT_d.ap()
                eng.dma_start(t[:], s[0:P, w * 4096:(w + 1) * 4096])
                xvw[w] = t

            # ---- prolog DMAs: four parallel queues, each in deadline order.
            # sync: xk stream (feeds the scores/exp cadence directly)
            # gpsimd: wq/xq (qproj), vector: wv/xv (vproj), scalar: wout
            dma_w_slice(1, 0, 1, nc.sync)    # wk slice for p=0
            dma_xk_window(0)
            dma_w_slice(0, 0, 1, nc.gpsimd)  # wq slice for p=0
            dma_xq_window(0)
            dma_w_slice(2, 0, 4, nc.scalar)  # full wv
            dma_xk_window(1)
            dma_xv_window(0, nc.scalar)
            dma_w_slice(1, 1, 4, nc.gpsimd)  # rest of wk
            dma_xv_window(1, nc.scalar)
            dma_xk_window(2)
            dma_w_slice(0, 1, 4, nc.gpsimd)  # rest of wq
            dma_xv_window(2, nc.gpsimd)
            dma_xk_window(3)
            dma_xv_window(3, nc.sync)
            dma_wout()

            # AV stationary pattern: zero the non-v columns, ones at the
            # denominator columns (64, 160 of each 256-block)
            one = int(np.float32(1.0).astype(NP_BF16).view(np.uint16))
            for s in range(SC):
                t = vst[s]
                nc.gpsimd._memset_packed(
                    bass.AP(t.tensor, t.offset + 64,
                            [[1024, 128], [256, 4], [1, 128]]), 0)
                nc.gpsimd._memset_packed(
                    bass.AP(t.tensor, t.offset + 64, [[1024, 128], [256, 4]]),
                    one)
                nc.gpsimd._memset_packed(
                    bass.AP(t.tensor, t.offset + 160, [[1024, 128], [256, 4]]),
                    one)

            # ---- projection bursts ----
            def kproj_into(p, w, ps, b):
                for ec in range(EC):
                    c = wk_col(ec, p)
                    nc.tensor.matmul(
                        ps[:, b * 512:b * 512 + 512],
                        wqA[:, c:c + P],
                        xkA[:, w * 4096 + ec * 512:w * 4096 + (ec + 1) * 512],
                        start=(ec == 0), stop=(ec == EC - 1))
                nc.vector.tensor_copy(
                    kT[p][:, w * 512:(w + 1) * 512], ps[:, b * 512:b * 512 + 512])

            def kproj_chunk(p, w):
                ps = psp.tile([P, 512 * GRP], F32, tag="sc", name="kps")
                kproj_into(p, w, ps, 0)

            def kproj_pair(p, w0):
                ps = psp.tile([P, 512 * GRP], F32, tag="sc", name="kps")
                kproj_into(p, w0, ps, 0)
                kproj_into(p, w0 + 1, ps, 1)

            def qproj_into(p, lg, ps, b):
                t = xqwin[lg]
                for ec in range(EC):
                    c = wq_col(ec, p)
                    nc.tensor.matmul(
                        ps[:, b * 512:b * 512 + 512],
                        wqA[:, c:c + P],
                        t[:, ec * 512:(ec + 1) * 512],
                        start=(ec == 0), stop=(ec == EC - 1))
                nc.vector.tensor_copy(
                    qT[p][:, lg * 512:(lg + 1) * 512], ps[:, b * 512:b * 512 + 512])

            def qproj(p, lg):
                ps = psp.tile([P, 512 * GRP], F32, tag="sc", name="qps")
                qproj_into(p, lg, ps, 0)

            def vproj_into(lt, ps, b):
                t = xvw[lt // 4]
                for ec in range(EC):
                    nc.tensor.matmul(
                        ps[:, b * 512:b * 512 + 512],
                        t[:, ec * 512 + (lt % 4) * P:ec * 512 + (lt % 4 + 1) * P],
                        wqA[:, wv_col(ec):wv_col(ec) + FG],
                        start=(ec == 0), stop=(ec == EC - 1))
                # strided drains: h0 dims -> cols {0:64}+256p, h1 -> {192:256}+256p
                psw = 512 * GRP
                dst0 = bass.AP(vst[lt].tensor, vst[lt].offset,
                               [[1024, 128], [256, 4], [1, 64]])
                src0 = bass.AP(ps.tensor, ps.offset + b * 512,
                               [[psw, 128], [128, 4], [1, 64]])
                nc.vector.tensor_copy(dst0, src0)
                dst1 = bass.AP(vst[lt].tensor, vst[lt].offset + 192,
                               [[1024, 128], [256, 4], [1, 64]])
                src1 = bass.AP(ps.tensor, ps.offset + b * 512 + 64,
                               [[psw, 128], [128, 4], [1, 64]])
                nc.vector.tensor_copy(dst1, src1)

            def vproj_pair(lt0):
                ps = psp.tile([P, 512 * GRP], F32, tag="sc", name="vps")
                vproj_into(lt0, ps, 0)
                vproj_into(lt0 + 1, ps, 1)

            def outproj_mms(lg, lt, ps, b, e0, e1):
                t0 = lg * 512 + lt * P
                for ec in range(e0, e1):
                    nc.tensor.matmul(
                        ps[:, b * 512:b * 512 + 512], avN[ec][:, t0:t0 + P],
                        woA[:, ec * E:ec * E + 512],
                        start=(ec == 0), stop=(ec == FT - 1))
                    nc.tensor.matmul(
                        ps[:, b * 512 + 512:b * 512 + 1024],
                        avN[ec][:, t0:t0 + P],
                        woA[:, ec * E + 512:ec * E + 1024],
                        start=(ec == 0), stop=(ec == FT - 1))

            def outproj_drain(lg, lt, ps, b):
                t0 = lg * 512 + lt * P
                osb = stage.tile([P, E], BF16, tag="osb", name="osb", bufs=2)
                nc.vector.tensor_copy(osb[:], ps[:, b * 512:b * 512 + 1024])
                nc.sync.dma_start(out_d.ap()[t0:t0 + P, :], osb[:])

            def outproj_into(lg, lt, ps, b):
                outproj_mms(lg, lt, ps, b, 0, FT)
                outproj_drain(lg, lt, ps, b)

            def outproj(lg, lt):
                ps = psp.tile([P, 512 * GRP], F32, tag="sc", name="ops")
                outproj_into(lg, lt, ps, 0)

            DEMOTE = 25

            def demoted(fn):
                # burst matmuls become gap-fillers: the Tile scheduler's
                # per-engine ready-heap prefers the next score groups, so
                # projections run in PE slack instead of ahead of scores
                def run():
                    with tc.high_priority(offset=-DEMOTE):
                        fn()
                return run

            pending_tails = []

            def attention_iter(p, lg, sched):
                avA = psav.tile([P, 512], F32, tag="avA", name="avA")
                avB = psav.tile([P, 512], F32, tag="avB", name="avB")
                av_bank = (avA, avB)

                def av_mms(t0, n, aT, p=p, av_bank=av_bank):
                    for j in range(n):
                        sc, h = divmod(t0 + j, 2)
                        nc.tensor.matmul(
                            av_bank[h][:],
                            vst[sc][:, p * 256 + 128 * h:p * 256 + 128 * h + 128],
                            aT[:, j * 512:(j + 1) * 512],
                            start=(sc == 0), stop=(sc == SC - 1))

                # score groups emitted in PAIRS: the 3+3 score matmuls of
                # two consecutive groups alternate h perfectly, so all three
                # adjacent (h0,h1) pairs run concurrently in the PE's two
                # 64-row tiles; the full-array AV matmuls come after.
                pending = []

                def flush_pending():
                    for t0_, n_, aT_ in pending:
                        av_mms(t0_, n_, aT_)
                    del pending[:]

                groups = list(range(0, NU, GRP))
                gi = 0
                while gi < len(groups):
                    blk = groups[gi:gi + 2]
                    aTs = []
                    for t0 in blk:
                        n = min(GRP, NU - t0)
                        ps = psp.tile([P, 512 * GRP], F32, tag="sc",
                                      name="scp")
                        for j in range(n):
                            sc, h = divmod(t0 + j, 2)
                            nc.tensor.matmul(
                                ps[:, j * 512:(j + 1) * 512],
                                kT[p][64 * h:64 * h + 64, sc * P:(sc + 1) * P],
                                qT[p][64 * h:64 * h + 64,
                                      lg * 512:(lg + 1) * 512],
                                start=True, stop=True)
                        aT = stage.tile([P, 512 * GRP], BF16, tag="aT",
                                        name="aT", bufs=5)
                        nc.scalar.activation(aT[:, 0:512 * n],
                                             ps[:, 0:512 * n],
                                             EXP, scale=0.125)
                        aTs.append((t0, n, aT))
                    if gi == 0 and pending_tails:
                        pending_tails.pop()()  # prev iter's tail after this
                        # iter's first exp is queued
                    for g in blk:
                        for fn in sched.get(g // GRP, ()):
                            fn()
                    flush_pending()
                    pending.extend(aTs)
                    gi += 2

                def tail(avA=avA, avB=avB, p=p, lg=lg,
                         flush_pending=flush_pending):
                    flush_pending()
                    avS0 = stage.tile([P, 512], F32, tag="avS0", name="avS0",
                                      bufs=1)
                    nc.vector.tensor_copy(avS0[:], avA[:])
                    avS1 = stage.tile([P, 512], F32, tag="avS1", name="avS1",
                                      bufs=1)
                    nc.vector.tensor_copy(avS1[:], avB[:])
                    # both denominators on partitions 0/32 -> one reciprocal
                    nc.vector.tensor_copy(rr[0:1, :], avS0[64:65, :])
                    nc.vector.tensor_copy(rr[32:33, :], avS1[32:33, :])
                    nc.vector.reciprocal(rr[0:64, :], rr[0:64, :])
                    r1 = stage.tile([1, 512], F32, tag="r1", name="r1", bufs=1)
                    nc.vector.tensor_copy(r1[0:1, :], rr[32:33, :])
                    bc0 = stage.tile([P, 512], F32, tag="bc0", name="bc0",
                                     bufs=1)
                    nc.gpsimd.partition_broadcast(bc0[:], rr[0:1, :])
                    bc1 = stage.tile([P, 512], F32, tag="bc1", name="bc1",
                                     bufs=1)
                    nc.gpsimd.partition_broadcast(bc1[:], r1[0:1, :])
                    nc.vector.tensor_mul(
                        avN[p][0:64, lg * 512:(lg + 1) * 512],
                        avS0[0:64, :], bc0[0:64, :])
                    nc.vector.tensor_mul(
                        avN[p][64:128, lg * 512:(lg + 1) * 512],
                        avS1[64:128, :], bc1[64:128, :])
                pending_tails.append(tail)

            # ---- prolog compute (overlapped with the DMA stream) ----
            kproj_chunk(0, 0)
            qproj(0, 0)

            # ---- the fused schedule (all bursts demoted) ----
            def make_sched(lg, p):
                D = demoted
                sched = {}
                if lg == 0 and p == 0:
                    sched[0] = [D(lambda: vproj_pair(0))]
                    sched[1] = [D(lambda: kproj_chunk(0, 1)),
                                D(lambda: vproj_pair(2))]
                    sched[2] = [D(lambda: vproj_pair(4))]
                    sched[3] = [D(lambda: kproj_chunk(0, 2))]
                    sched[4] = [D(lambda: vproj_pair(6))]
                    sched[5] = [D(lambda: kproj_chunk(0, 3))]
                    sched[6] = [D(lambda: vproj_pair(8))]
                    sched[7] = [D(lambda: vproj_pair(10))]
                    sched[8] = [D(lambda: kproj_chunk(1, 0)),
                                D(lambda: vproj_pair(12))]
                    sched[9] = [D(lambda: qproj(1, 0)),
                                D(lambda: vproj_pair(14))]
                    return sched
                if lg == 0 and p == 1:
                    sched[0] = [D(lambda: kproj_chunk(1, 1))]
                    sched[2] = [D(lambda: kproj_chunk(1, 2))]
                    sched[4] = [D(lambda: kproj_chunk(1, 3))]
                    sched[6] = [D(lambda: kproj_chunk(2, 0))]
                    sched[8] = [D(lambda: qproj(2, 0))]
                    return sched
                if lg == 0 and p == 2:
                    sched[0] = [D(lambda: kproj_chunk(2, 1))]
                    sched[2] = [D(lambda: kproj_chunk(2, 2))]
                    sched[4] = [D(lambda: kproj_chunk(2, 3))]
                    sched[6] = [D(lambda: kproj_chunk(3, 0))]
                    sched[8] = [D(lambda: qproj(3, 0))]
                    return sched
                if lg == 0 and p == 3:
                    sched[0] = [D(lambda: kproj_chunk(3, 1))]
                    sched[2] = [D(lambda: kproj_chunk(3, 2))]
                    sched[4] = [D(lambda: kproj_chunk(3, 3))]
                    sched[6] = [lambda: dma_xq_window(1)]
                    sched[8] = [D(lambda: qproj(0, 1))]
                    return sched
                # steady state (lg >= 1): qproj-next and outproj-prev bursts
                if p < 3:
                    sched[4] = [D(lambda: qproj(p + 1, lg))]
                elif lg < 3:
                    sched[2] = [lambda: dma_xq_window(lg + 1)]
                    sched[4] = [D(lambda: qproj(0, lg + 1))]
                sched[7] = [D(lambda: outproj(lg - 1, p))]
                return sched

            for lg in range(LG):
                for p in range(FT):
                    attention_iter(p, lg, make_sched(lg, p))
            # endgame: avN[0..2] for lg=3 are done; start the final outprojs'
            # first 3 contraction steps now so the PE stays warm while the
            # last tail's reciprocal chain runs, then finish with avN[3].
            psX = psp.tile([P, 512 * GRP], F32, tag="sc", name="endA")
            outproj_mms(3, 0, psX, 0, 0, FT - 1)
            psY = psp.tile([P, 512 * GRP], F32, tag="sc", name="endB")
            outproj_mms(3, 1, psY, 0, 0, FT - 1)
            while pending_tails:
                pending_tails.pop()()
            outproj_mms(3, 0, psX, 0, FT - 1, FT)
            outproj_drain(3, 0, psX, 0)
            outproj_mms(3, 1, psY, 0, FT - 1, FT)
            outproj_drain(3, 1, psY, 0)
            for lt in (2, 3):
                outproj(3, lt)

    nc.compile()
    return nc


_NC = None


def _get_nc():
    global _NC
    if _NC is None:
        _NC = _build()
    return _NC


def _x_hw(xT):
    # [E, L] fp32 (already transposed) -> [P, LG*EC*512] window-major bf16:
    # col = w*4096 + ec*512 + c  maps to  xT[ec*128+p, w*512+c]
    a = xT.reshape(EC, P, LG, 512)          # (ec, p, w, c)
    a = a.transpose(1, 2, 0, 3)             # (p, w, ec, c)
    return np.ascontiguousarray(a.reshape(P, EC * L)).astype(NP_BF16)


def _shard_inputs(query, key, value, in_proj_weight, out_proj_weight):
    B = query.shape[0]
    # per-batch hw-layout bf16 activations (shared by the 2 cores per batch)
    xT = {}
    for b in range(B):
        xT[b] = tuple(
            _x_hw(np.asarray(x[b], dtype=np.float32).T)
            for x in (query, key, value))
    # per-head-group weight blocks, slice-major (see W_SL in _build)
    wblk = {}
    for g in range(2):
        sl = slice(FG * g, FG * g + FG)
        wq = in_proj_weight[0 * E:1 * E][sl].T    # [E, FG]
        wk = in_proj_weight[1 * E:2 * E][sl].T
        wv = in_proj_weight[2 * E:3 * E][sl].T

        def blk(w, c0, c1):
            # [E, c1-c0] -> (p, ec, c) ec-major rows
            a = w[:, c0:c1].reshape(EC, P, c1 - c0)
            return a.transpose(1, 0, 2).reshape(P, EC * (c1 - c0))

        wqh = np.concatenate([
            blk(wk, 0, P), blk(wq, 0, P),
            blk(wk, P, FG), blk(wq, P, FG),
            blk(wv, 0, FG)], axis=1)
        wqh = np.ascontiguousarray(wqh).astype(NP_BF16)
        wo = out_proj_weight[:, sl].T             # [FG, E]
        woh = np.ascontiguousarray(
            wo.reshape(FT, P, E).transpose(1, 0, 2).reshape(P, FT * E)
        ).astype(NP_BF16)
        wblk[g] = (wqh, woh)
    in_maps = []
    for c in range(8):
        b, g = divmod(c, 2)
        xq, xk, xv = xT[b]
        wqh, woh = wblk[g]
        in_maps.append({
            "xqh": xq, "xkh": xk, "xvh": xv,
            "wqh": wqh, "woh": woh,
        })
    return in_maps


def run_sharded(in_maps, **kwargs):
    nc = _get_nc()
    return bass_utils.run_bass_kernel_spmd(
        nc, in_maps, core_ids=list(range(8)), **kwargs)


def kernel(query, key, value, in_proj_weight, out_proj_weight, out_proj_bias):
    query = np.asarray(query, dtype=np.float32)
    key = np.asarray(key, dtype=np.float32)
    value = np.asarray(value, dtype=np.float32)
    in_proj_weight = np.asarray(in_proj_weight, dtype=np.float32)
    out_proj_weight = np.asarray(out_proj_weight, dtype=np.float32)
    out_proj_bias = np.asarray(out_proj_bias, dtype=np.float32)

    in_maps = _shard_inputs(query, key, value, in_proj_weight, out_proj_weight)
    res = run_sharded(in_maps)
    out = np.empty((4, L, E), dtype=np.float32)
    for b in range(4):
        out[b] = (np.asarray(res.results[2 * b]["out"], dtype=np.float32)
                  + np.asarray(res.results[2 * b + 1]["out"], dtype=np.float32))
    out += out_proj_bias
    return out
